# revision 59
# baseline (speedup 1.0000x reference)
"""Trainium2 Bass kernel for nn_Amplituedro (weighted embedding lookup).

path[b] = (sum_k w[b,k] * vertices[idx[b,k]]) / sum_k w[b,k]
eff     = mean_b ||path[b]||

Data-parallel over batch: 8 cores x 8192 rows. Per core, 8 groups of
8 x 128-row tiles with an interleaved row mapping (row = g*1024 + p*8 + t)
so per-partition DRAM accesses are contiguous (256B loads / 8KB stores).

Per group (prep) and per pair of 128-row tiles (software-pipelined: the
S stage for pair u+1 is emitted before the M stage of pair u, so the PE
never stalls on the transpose->copy->load-weights chain):
  - prep: one DMA load each for indices/weights [128,8,8]; DVE row totals
    -> reciprocals; normalized bf16 weights w' = w/total in one
    scalar_tensor_tensor with a zero-stride broadcast of the reciprocals;
    scatter offsets offs = 512*(t%2) + 64*k + idx (int16) in one int add
  - S: GPSIMD local_scatter builds one-hot rows
    eqw[b, (t%2)*512 + 64*k + e] = w'[b,k] (collision-free by k-slot);
    DVE 3-op bf16 add-tree reduces k -> agg2 [128, 2, 64]; PE transpose
    (identity matmul) -> psum; DVE copy -> aggT [128,128] bf16 SBUF
  - M: per tile h: PE matmuls path = aggT_h.T @ V (N=512, psum f32) and
    m1 = aggT_h.T @ G (G = V V^T, N=64); DVE scalar_tensor_tensor
    accumulates sqnorm[b] = sum_e m1[b,e]*agg[b,e] = ||path_b||^2; ACT
    copies psum -> SBUF bf16 (already normalized); one 1MB store per group
Epilogue: norm = sqrt(sqnorm); per-partition partial sums -> "eff".
Host: concat path shards (bf16 -> f32), sum efficiency partials / B.

Measured on 8 axon-tunneled TRN2 NeuronCores: ~77-95 us whole-NEFF
exec_time_ns (run-to-run device variance), path rel-err 2.9e-3,
efficiency rel-err 1e-4 (bf16 table/weights; f32 accumulation).
"""

import numpy as np

N_CORES = 8
B_FULL = 65536
B_CORE = B_FULL // N_CORES  # 8192
K = 8
E = 64
D = 512
GT = 8  # tiles per group
NG = B_CORE // (128 * GT)  # 8 groups
NT = B_CORE // 128  # 64 tiles

_CACHE = {}


def _build():
    import concourse.bacc as bacc
    import concourse.mybir as mybir
    import concourse.tile as tile
    from concourse import masks

    f32 = mybir.dt.float32
    bf16 = mybir.dt.bfloat16
    i32 = mybir.dt.int32
    i16 = mybir.dt.int16
    ALU = mybir.AluOpType
    AF = mybir.ActivationFunctionType

    nc = bacc.Bacc(None, target_bir_lowering=False, debug=False)

    idx_d = nc.declare_dram_parameter("expert_indices", [B_CORE, K], i32, isOutput=False)
    w_d = nc.declare_dram_parameter("expert_weights", [B_CORE, K], f32, isOutput=False)
    # vertices pre-cast to bf16 on host; stacked twice so both PE partition
    # halves hold a copy (lhsT base partition must match rhs base partition).
    v_d = nc.declare_dram_parameter("vertices_bf", [2, E, D], bf16, isOutput=False)
    vt_d = nc.declare_dram_parameter("vertices_t_bf", [D, E], bf16, isOutput=False)
    path_d = nc.declare_dram_parameter("path", [B_CORE, D], bf16, isOutput=True)
    eff_d = nc.declare_dram_parameter("eff", [128, 1], f32, isOutput=True)

    with tile.TileContext(nc) as tc:
        with (
            tc.tile_pool(name="const", bufs=1) as cpool,
            tc.tile_pool(name="work", bufs=6) as pool,
            tc.tile_pool(name="prep", bufs=NG) as prpool,
            tc.tile_pool(name="aggTp", bufs=8) as apool,
            tc.tile_pool(name="out", bufs=5) as opool,
            tc.tile_pool(name="ppair", bufs=2, space="PSUM") as ppool,
            tc.tile_pool(name="ptrans", bufs=2, space="PSUM") as tpool,
            tc.tile_pool(name="pm1", bufs=2, space="PSUM") as mpool,
        ):
            # ---- constants ----
            # block-diagonal tables: all three matmuls of a pair share one
            # full-array [128,128] aggT weight load.
            #   vblk[:, 0, :] = [V; 0]   vblk[:, 1, :] = [0; V]
            #   gblk          = diag(G, G)
            vblk = cpool.tile([128, 2, D], bf16)
            nc.gpsimd.memset(vblk[:], 0.0)
            nc.sync.dma_start(vblk[0:64, 0, :], v_d[0])
            nc.sync.dma_start(vblk[64:128, 1, :], v_d[1])
            vtb = cpool.tile([128, 4, E], bf16)
            nc.sync.dma_start(vtb[:], vt_d.rearrange("(c p) e -> p c e", p=128))

            # scatter offset bias first: the first local_scatter depends on it
            # (via offs), so keep it at the head of the gpsimd stream
            kvec = cpool.tile([128, GT, K], i32)
            nc.gpsimd.iota(
                kvec[:], pattern=[[0, GT // 2], [512, 2], [64, K]], base=0,
                channel_multiplier=0,
            )

            ident = cpool.tile([128, 128], bf16)
            masks.make_identity(nc, ident[:])

            # G = V @ V.T  [64, 64], replicated into both partition halves
            g_ps = ppool.tile([E, E], f32, tag="pps")
            for c in range(4):
                nc.tensor.matmul(
                    g_ps[:], vtb[:, c, :], vtb[:, c, :], start=(c == 0), stop=(c == 3)
                )
            gblk = cpool.tile([128, 128], bf16)
            nc.gpsimd.memset(gblk[:], 0.0)
            nc.vector.tensor_copy(gblk[0:64, 0:64], g_ps[:])
            nc.vector.tensor_copy(gblk[64:128, 64:128], g_ps[:])

            recips = cpool.tile([128, NT], f32)
            sqnorms = cpool.tile([128, NT], f32)

            NPAIR = NT // 2
            PPG = GT // 2  # pairs per group
            state = {}  # per-group tiles

            def group_prep(g):
                r0 = g * 128 * GT
                rows = slice(r0, r0 + 128 * GT)
                idx8 = prpool.tile([128, GT, K], i32)
                nc.sync.dma_start(idx8[:], idx_d[rows, :].rearrange("(p t) k -> p t k", p=128))
                w8 = prpool.tile([128, GT, K], f32)
                nc.sync.dma_start(w8[:], w_d[rows, :].rearrange("(p t) k -> p t k", p=128))

                tot8 = prpool.tile([128, GT], f32)
                nc.vector.tensor_reduce(tot8[:], w8[:], axis=mybir.AxisListType.X, op=ALU.add)
                nc.vector.reciprocal(recips[:, g * GT : (g + 1) * GT], tot8[:])

                # normalize during the bf16 cast: w' = w / total
                # (reciprocals broadcast over k via a zero-stride AP)
                w8b = prpool.tile([128, GT, K], bf16)
                rb = recips[:, g * GT : (g + 1) * GT].unsqueeze(2).broadcast_to([128, GT, K])
                nc.vector.scalar_tensor_tensor(
                    out=w8b[:], in0=w8[:], scalar=1.0, in1=rb,
                    op0=ALU.mult, op1=ALU.mult,
                )
                offs = prpool.tile([128, GT, K], i16)
                nc.vector.tensor_tensor(offs[:], idx8[:], kvec[:], op=ALU.add)
                state[g] = {"w8b": w8b, "offs": offs}

            qstate = {}  # quad-index -> eqw tile

            def stage_s(up):
                """scatter for pair up; on quad completion: tree + 2x transpose"""
                g, u = divmod(up, PPG)
                st = state[g]
                pair = slice(2 * u, 2 * u + 2)
                q, v = divmod(up, 2)
                if v == 0:
                    eqw = pool.tile([128, 4, 512], bf16)
                    qstate[q] = eqw
                else:
                    eqw = qstate[q]
                nc.gpsimd.local_scatter(
                    eqw[:, 2 * v : 2 * v + 2, :].rearrange("p t c -> p (t c)"),
                    st["w8b"][:, pair, :].rearrange("p t k -> p (t k)"),
                    st["offs"][:, pair, :].rearrange("p t k -> p (t k)"),
                    channels=128,
                    num_elems=1024,
                    num_idxs=16,
                )
                if v == 0:
                    return
                # quad complete: k-reduction tree across all 4 tiles at once
                eqw = qstate.pop(q)
                s1 = pool.tile([128, 4, 256], bf16)
                nc.vector.tensor_tensor(s1[:], eqw[:, :, 0:256], eqw[:, :, 256:512], op=ALU.add)
                s2 = pool.tile([128, 4, 128], bf16)
                nc.vector.tensor_tensor(s2[:], s1[:, :, 0:128], s1[:, :, 128:256], op=ALU.add)
                agg4 = pool.tile([128, 4, E], bf16)
                nc.vector.tensor_tensor(agg4[:], s2[:, :, 0:64], s2[:, :, 64:128], op=ALU.add)

                for w in range(2):
                    pu = 2 * q + w
                    gg, uu = divmod(pu, PPG)
                    agg2 = agg4[:, 2 * w : 2 * w + 2, :]
                    psT = tpool.tile([128, 128], bf16)
                    nc.tensor.transpose(psT[:], agg2.rearrange("p t e -> p (t e)"), ident[:])
                    aggT = apool.tile([128, 128], bf16)
                    nc.vector.tensor_copy(aggT[:], psT[:])
                    state[gg][uu] = (aggT, agg2)

            def stage_m(up):
                """matmuls + sqnorm + psum->sbuf copy (+ store at group end)"""
                g, u = divmod(up, PPG)
                st = state[g]
                aggT, agg2 = st.pop(u)
                pair = slice(2 * u, 2 * u + 2)
                if u == 0:
                    osb = opool.tile([128, GT, D], bf16, name="osb", tag="osb")
                    st["osb"] = osb
                osb = st["osb"]
                pps = ppool.tile([128, 2, D], f32, tag="pps")
                m1 = mpool.tile([128, 128], f32)
                # all three matmuls share the same full-array stationary aggT
                # (one weight load); block-diagonal rhs keeps tiles separate.
                # NOTE: do NOT split these into half-array matmuls on disjoint
                # row-groups — that crashes the device deterministically.
                nc.tensor.matmul(pps[:, 0, :], aggT[:], vblk[:, 0, :], start=True, stop=True)
                nc.tensor.matmul(pps[:, 1, :], aggT[:], vblk[:, 1, :], start=True, stop=True)
                nc.tensor.matmul(m1[:], aggT[:], gblk[:], start=True, stop=True)
                for h in range(2):
                    col = g * GT + 2 * u + h
                    scr = pool.tile([128, E], bf16)
                    nc.vector.scalar_tensor_tensor(
                        out=scr[:],
                        in0=m1[:, 64 * h : 64 * h + 64],
                        scalar=1.0,
                        in1=agg2[:, h, :],
                        op0=ALU.mult,
                        op1=ALU.mult,
                        accum_out=sqnorms[:, col : col + 1],
                    )
                # agg already normalized -> plain pair-wide psum->sbuf copy
                nc.scalar.activation(osb[:, pair, :], pps[:], AF.Copy, bias=0.0)
                if u % 2 == 1:
                    # store half a group (4 tiles = 4KB contiguous/partition)
                    v = u // 2
                    r0 = g * 128 * GT
                    rows = slice(r0, r0 + 128 * GT)
                    nc.sync.dma_start(
                        path_d[rows, :].rearrange("(p t) d -> p t d", p=128)[
                            :, 4 * v : 4 * v + 4, :
                        ],
                        osb[:, 4 * v : 4 * v + 4, :],
                    )
                    if u == PPG - 1:
                        del state[g]["osb"]

            # software pipeline: S runs two pairs ahead of M
            LOOKAHEAD = 2
            group_prep(0)
            for p0 in range(LOOKAHEAD):
                if p0 % PPG == 0 and p0 > 0:
                    group_prep(p0 // PPG)
                stage_s(p0)
            for up in range(NPAIR):
                nu = up + LOOKAHEAD
                if nu < NPAIR:
                    if nu % PPG == 0:
                        group_prep(nu // PPG)
                    stage_s(nu)
                stage_m(up)

            # ---- efficiency epilogue ----
            # agg was pre-normalized, so sqnorms[b] is already ||path_b||^2
            norms = cpool.tile([128, NT], f32)
            nc.scalar.activation(norms[:], sqnorms[:], AF.Sqrt)
            effp = cpool.tile([128, 1], f32)
            nc.vector.tensor_reduce(effp[:], norms[:], axis=mybir.AxisListType.X, op=ALU.add)
            nc.sync.dma_start(eff_d[:], effp[:])

    nc.compile()
    return nc


def _get_nc():
    if "nc" not in _CACHE:
        _CACHE["nc"] = _build()
    return _CACHE["nc"]


def _run(in_maps, trace=False):
    from concourse.bass_utils import run_bass_kernel_spmd

    nc = _get_nc()
    return run_bass_kernel_spmd(nc, in_maps, list(range(N_CORES)), trace=trace)


def _make_in_maps(expert_indices, expert_weights, vertices):
    import ml_dtypes

    idx = np.ascontiguousarray(np.asarray(expert_indices, dtype=np.int32))
    w = np.ascontiguousarray(np.asarray(expert_weights, dtype=np.float32))
    v = np.asarray(vertices, dtype=np.float32)
    vbf = np.ascontiguousarray(
        np.broadcast_to(v.astype(ml_dtypes.bfloat16), (2, E, D))
    )
    vtbf = np.ascontiguousarray(v.T.astype(ml_dtypes.bfloat16))
    in_maps = []
    for i in range(N_CORES):
        s = slice(i * B_CORE, (i + 1) * B_CORE)
        in_maps.append(
            {
                "expert_indices": np.ascontiguousarray(idx[s]),
                "expert_weights": np.ascontiguousarray(w[s]),
                "vertices_bf": vbf,
                "vertices_t_bf": vtbf,
            }
        )
    return in_maps


def _assemble(results):
    path = np.concatenate(
        [np.asarray(r["path"]).astype(np.float32) for r in results], axis=0
    )
    eff = sum(float(np.asarray(r["eff"], dtype=np.float64).sum()) for r in results)
    eff = np.float32(eff / B_FULL)
    return path, eff


def kernel(expert_indices, expert_weights, vertices):
    in_maps = _make_in_maps(expert_indices, expert_weights, vertices)
    last_err = None
    for attempt in range(3):
        try:
            res = _run(in_maps, trace=False)
            return _assemble(res.results)
        except Exception as e:  # rare transient device errors -> retry
            last_err = e
            _CACHE.clear()
    raise last_err


# revision 60
# speedup vs baseline: 1.0314x; 1.0314x over previous
"""Trainium2 Bass kernel for nn_Amplituedro (weighted embedding lookup).

path[b] = (sum_k w[b,k] * vertices[idx[b,k]]) / sum_k w[b,k]
eff     = mean_b ||path[b]||

Data-parallel over batch: 8 cores x 8192 rows. Per core, 8 groups of
8 x 128-row tiles with an interleaved row mapping (row = g*1024 + p*8 + t)
so per-partition DRAM accesses are contiguous (256B loads / 8KB stores).

Per group (prep) and per pair of 128-row tiles (software-pipelined: the
S stage for pair u+1 is emitted before the M stage of pair u, so the PE
never stalls on the transpose->copy->load-weights chain):
  - prep: one DMA load each for indices/weights [128,8,8]; DVE row totals
    -> reciprocals; normalized bf16 weights w' = w/total in one
    scalar_tensor_tensor with a zero-stride broadcast of the reciprocals;
    scatter offsets offs = 512*(t%2) + 64*k + idx (int16) in one int add
  - S: GPSIMD local_scatter builds one-hot rows
    eqw[b, (t%2)*512 + 64*k + e] = w'[b,k] (collision-free by k-slot);
    DVE 3-op bf16 add-tree reduces k -> agg2 [128, 2, 64]; PE transpose
    (identity matmul) -> psum; DVE copy -> aggT [128,128] bf16 SBUF
  - M: per tile h: PE matmuls path = aggT_h.T @ V (N=512, psum f32) and
    m1 = aggT_h.T @ G (G = V V^T, N=64); DVE scalar_tensor_tensor
    accumulates sqnorm[b] = sum_e m1[b,e]*agg[b,e] = ||path_b||^2; ACT
    copies psum -> SBUF bf16 (already normalized); one 1MB store per group
Epilogue: norm = sqrt(sqnorm); per-partition partial sums -> "eff".
Host: concat path shards (bf16 -> f32), sum efficiency partials / B.

Measured on 8 axon-tunneled TRN2 NeuronCores: ~77-95 us whole-NEFF
exec_time_ns (run-to-run device variance), path rel-err 2.9e-3,
efficiency rel-err 1e-4 (bf16 table/weights; f32 accumulation).
"""

import numpy as np

N_CORES = 8
B_FULL = 65536
B_CORE = B_FULL // N_CORES  # 8192
K = 8
E = 64
D = 512
GT = 8  # tiles per group
NG = B_CORE // (128 * GT)  # 8 groups
NT = B_CORE // 128  # 64 tiles

_CACHE = {}


def _build():
    import concourse.bacc as bacc
    import concourse.mybir as mybir
    import concourse.tile as tile
    from concourse import masks

    f32 = mybir.dt.float32
    bf16 = mybir.dt.bfloat16
    i32 = mybir.dt.int32
    i16 = mybir.dt.int16
    ALU = mybir.AluOpType
    AF = mybir.ActivationFunctionType

    nc = bacc.Bacc(None, target_bir_lowering=False, debug=False)

    idx_d = nc.declare_dram_parameter("expert_indices", [B_CORE, K], i32, isOutput=False)
    w_d = nc.declare_dram_parameter("expert_weights", [B_CORE, K], f32, isOutput=False)
    # vertices pre-cast to bf16 on host; stacked twice so both PE partition
    # halves hold a copy (lhsT base partition must match rhs base partition).
    v_d = nc.declare_dram_parameter("vertices_bf", [2, E, D], bf16, isOutput=False)
    vt_d = nc.declare_dram_parameter("vertices_t_bf", [D, E], bf16, isOutput=False)
    path_d = nc.declare_dram_parameter("path", [B_CORE, D], bf16, isOutput=True)
    eff_d = nc.declare_dram_parameter("eff", [128, 1], f32, isOutput=True)

    with tile.TileContext(nc) as tc:
        with (
            tc.tile_pool(name="const", bufs=1) as cpool,
            tc.tile_pool(name="work", bufs=6) as pool,
            tc.tile_pool(name="prep", bufs=NG) as prpool,
            tc.tile_pool(name="aggTp", bufs=8) as apool,
            tc.tile_pool(name="out", bufs=5) as opool,
            tc.tile_pool(name="ppair", bufs=2, space="PSUM") as ppool,
            tc.tile_pool(name="ptrans", bufs=2, space="PSUM") as tpool,
            tc.tile_pool(name="pm1", bufs=2, space="PSUM") as mpool,
        ):
            # ---- constants ----
            # block-diagonal tables: all three matmuls of a pair share one
            # full-array [128,128] aggT weight load.
            #   vblk[:, 0, :] = [V; 0]   vblk[:, 1, :] = [0; V]
            #   gblk          = diag(G, G)
            vblk = cpool.tile([128, 2, D], bf16)
            nc.gpsimd.memset(vblk[:], 0.0)
            nc.sync.dma_start(vblk[0:64, 0, :], v_d[0])
            nc.sync.dma_start(vblk[64:128, 1, :], v_d[1])
            vtb = cpool.tile([128, 4, E], bf16)
            nc.sync.dma_start(vtb[:], vt_d.rearrange("(c p) e -> p c e", p=128))

            # scatter offset bias first: the first local_scatter depends on it
            # (via offs), so keep it at the head of the gpsimd stream
            kvec = cpool.tile([128, GT, K], i32)
            nc.gpsimd.iota(
                kvec[:], pattern=[[0, GT // 2], [512, 2], [64, K]], base=0,
                channel_multiplier=0,
            )

            ident = cpool.tile([128, 128], bf16)
            masks.make_identity(nc, ident[:])

            # G = V @ V.T  [64, 64], replicated into both partition halves
            g_ps = ppool.tile([E, E], f32, tag="pps")
            for c in range(4):
                nc.tensor.matmul(
                    g_ps[:], vtb[:, c, :], vtb[:, c, :], start=(c == 0), stop=(c == 3)
                )
            gblk = cpool.tile([128, 128], bf16)
            nc.gpsimd.memset(gblk[:], 0.0)
            nc.vector.tensor_copy(gblk[0:64, 0:64], g_ps[:])
            nc.vector.tensor_copy(gblk[64:128, 64:128], g_ps[:])

            recips = cpool.tile([128, NT], f32)
            sqnorms = cpool.tile([128, NT], f32)

            NPAIR = NT // 2
            PPG = GT // 2  # pairs per group
            state = {}  # per-group tiles

            def group_prep(g):
                r0 = g * 128 * GT
                rows = slice(r0, r0 + 128 * GT)
                idx8 = prpool.tile([128, GT, K], i32)
                nc.sync.dma_start(idx8[:], idx_d[rows, :].rearrange("(p t) k -> p t k", p=128))
                w8 = prpool.tile([128, GT, K], f32)
                nc.sync.dma_start(w8[:], w_d[rows, :].rearrange("(p t) k -> p t k", p=128))

                tot8 = prpool.tile([128, GT], f32)
                nc.vector.tensor_reduce(tot8[:], w8[:], axis=mybir.AxisListType.X, op=ALU.add)
                nc.vector.reciprocal(recips[:, g * GT : (g + 1) * GT], tot8[:])

                # normalize during the bf16 cast: w' = w / total
                # (reciprocals broadcast over k via a zero-stride AP)
                w8b = prpool.tile([128, GT, K], bf16)
                rb = recips[:, g * GT : (g + 1) * GT].unsqueeze(2).broadcast_to([128, GT, K])
                nc.vector.scalar_tensor_tensor(
                    out=w8b[:], in0=w8[:], scalar=1.0, in1=rb,
                    op0=ALU.mult, op1=ALU.mult,
                )
                offs = prpool.tile([128, GT, K], i16)
                nc.vector.tensor_tensor(offs[:], idx8[:], kvec[:], op=ALU.add)
                state[g] = {"w8b": w8b, "offs": offs}

            qstate = {}  # quad-index -> eqw tile

            def stage_s(up):
                """scatter for pair up; on quad completion: tree + 2x transpose"""
                g, u = divmod(up, PPG)
                st = state[g]
                pair = slice(2 * u, 2 * u + 2)
                q, v = divmod(up, 2)
                if v == 0:
                    eqw = pool.tile([128, 4, 512], bf16)
                    qstate[q] = eqw
                else:
                    eqw = qstate[q]
                nc.gpsimd.local_scatter(
                    eqw[:, 2 * v : 2 * v + 2, :].rearrange("p t c -> p (t c)"),
                    st["w8b"][:, pair, :].rearrange("p t k -> p (t k)"),
                    st["offs"][:, pair, :].rearrange("p t k -> p (t k)"),
                    channels=128,
                    num_elems=1024,
                    num_idxs=16,
                )
                if v == 0:
                    return
                # quad complete: k-reduction tree across all 4 tiles at once
                eqw = qstate.pop(q)
                s1 = pool.tile([128, 4, 256], bf16)
                nc.vector.tensor_tensor(s1[:], eqw[:, :, 0:256], eqw[:, :, 256:512], op=ALU.add)
                s2 = pool.tile([128, 4, 128], bf16)
                nc.vector.tensor_tensor(s2[:], s1[:, :, 0:128], s1[:, :, 128:256], op=ALU.add)
                agg4 = pool.tile([128, 4, E], bf16)
                nc.vector.tensor_tensor(agg4[:], s2[:, :, 0:64], s2[:, :, 64:128], op=ALU.add)

                for w in range(2):
                    pu = 2 * q + w
                    gg, uu = divmod(pu, PPG)
                    agg2 = agg4[:, 2 * w : 2 * w + 2, :]
                    psT = tpool.tile([128, 128], bf16)
                    nc.tensor.transpose(psT[:], agg2.rearrange("p t e -> p (t e)"), ident[:])
                    aggT = apool.tile([128, 128], bf16)
                    nc.vector.tensor_copy(aggT[:], psT[:])
                    state[gg][uu] = (aggT, agg2)

            def stage_m(up):
                """matmuls + sqnorm + psum->sbuf copy (+ store at group end)"""
                g, u = divmod(up, PPG)
                st = state[g]
                aggT, agg2 = st.pop(u)
                pair = slice(2 * u, 2 * u + 2)
                if u == 0:
                    osb = opool.tile([128, GT, D], bf16, name="osb", tag="osb")
                    st["osb"] = osb
                osb = st["osb"]
                pps = ppool.tile([128, 2, D], f32, tag="pps")
                m1 = mpool.tile([128, 128], f32)
                # all three matmuls share the same full-array stationary aggT
                # (one weight load); block-diagonal rhs keeps tiles separate.
                # NOTE: do NOT split these into half-array matmuls on disjoint
                # row-groups — that crashes the device deterministically.
                nc.tensor.matmul(pps[:, 0, :], aggT[:], vblk[:, 0, :], start=True, stop=True)
                nc.tensor.matmul(pps[:, 1, :], aggT[:], vblk[:, 1, :], start=True, stop=True)
                nc.tensor.matmul(m1[:], aggT[:], gblk[:], start=True, stop=True)
                for h in range(2):
                    col = g * GT + 2 * u + h
                    scr = pool.tile([128, E], bf16)
                    nc.vector.scalar_tensor_tensor(
                        out=scr[:],
                        in0=m1[:, 64 * h : 64 * h + 64],
                        scalar=1.0,
                        in1=agg2[:, h, :],
                        op0=ALU.mult,
                        op1=ALU.mult,
                        accum_out=sqnorms[:, col : col + 1],
                    )
                # agg already normalized -> plain pair-wide psum->sbuf copy
                nc.scalar.activation(osb[:, pair, :], pps[:], AF.Copy, bias=0.0)
                if u % 2 == 1:
                    # store half a group (4 tiles = 4KB contiguous/partition)
                    v = u // 2
                    r0 = g * 128 * GT
                    rows = slice(r0, r0 + 128 * GT)
                    nc.sync.dma_start(
                        path_d[rows, :].rearrange("(p t) d -> p t d", p=128)[
                            :, 4 * v : 4 * v + 4, :
                        ],
                        osb[:, 4 * v : 4 * v + 4, :],
                    )
                    if u == PPG - 1:
                        del state[g]["osb"]

            # software pipeline: S runs three pairs ahead of M
            LOOKAHEAD = 3
            group_prep(0)
            for p0 in range(LOOKAHEAD):
                if p0 % PPG == 0 and p0 > 0:
                    group_prep(p0 // PPG)
                stage_s(p0)
            for up in range(NPAIR):
                nu = up + LOOKAHEAD
                if nu < NPAIR:
                    if nu % PPG == 0:
                        group_prep(nu // PPG)
                    stage_s(nu)
                stage_m(up)

            # ---- efficiency epilogue ----
            # agg was pre-normalized, so sqnorms[b] is already ||path_b||^2
            norms = cpool.tile([128, NT], f32)
            nc.scalar.activation(norms[:], sqnorms[:], AF.Sqrt)
            effp = cpool.tile([128, 1], f32)
            nc.vector.tensor_reduce(effp[:], norms[:], axis=mybir.AxisListType.X, op=ALU.add)
            nc.sync.dma_start(eff_d[:], effp[:])

    nc.compile()
    return nc


def _get_nc():
    if "nc" not in _CACHE:
        _CACHE["nc"] = _build()
    return _CACHE["nc"]


def _run(in_maps, trace=False):
    from concourse.bass_utils import run_bass_kernel_spmd

    nc = _get_nc()
    return run_bass_kernel_spmd(nc, in_maps, list(range(N_CORES)), trace=trace)


def _make_in_maps(expert_indices, expert_weights, vertices):
    import ml_dtypes

    idx = np.ascontiguousarray(np.asarray(expert_indices, dtype=np.int32))
    w = np.ascontiguousarray(np.asarray(expert_weights, dtype=np.float32))
    v = np.asarray(vertices, dtype=np.float32)
    vbf = np.ascontiguousarray(
        np.broadcast_to(v.astype(ml_dtypes.bfloat16), (2, E, D))
    )
    vtbf = np.ascontiguousarray(v.T.astype(ml_dtypes.bfloat16))
    in_maps = []
    for i in range(N_CORES):
        s = slice(i * B_CORE, (i + 1) * B_CORE)
        in_maps.append(
            {
                "expert_indices": np.ascontiguousarray(idx[s]),
                "expert_weights": np.ascontiguousarray(w[s]),
                "vertices_bf": vbf,
                "vertices_t_bf": vtbf,
            }
        )
    return in_maps


def _assemble(results):
    path = np.concatenate(
        [np.asarray(r["path"]).astype(np.float32) for r in results], axis=0
    )
    eff = sum(float(np.asarray(r["eff"], dtype=np.float64).sum()) for r in results)
    eff = np.float32(eff / B_FULL)
    return path, eff


def kernel(expert_indices, expert_weights, vertices):
    in_maps = _make_in_maps(expert_indices, expert_weights, vertices)
    last_err = None
    for attempt in range(3):
        try:
            res = _run(in_maps, trace=False)
            return _assemble(res.results)
        except Exception as e:  # rare transient device errors -> retry
            last_err = e
            _CACHE.clear()
    raise last_err


# revision 64
# speedup vs baseline: 1.0453x; 1.0135x over previous
"""Trainium2 Bass kernel for nn_Amplituedro (weighted embedding lookup).

path[b] = (sum_k w[b,k] * vertices[idx[b,k]]) / sum_k w[b,k]
eff     = mean_b ||path[b]||

Data-parallel over batch: 8 cores x 8192 rows. Per core, 8 groups of
8 x 128-row tiles with an interleaved row mapping (row = g*1024 + p*8 + t)
so per-partition DRAM accesses are contiguous (256B loads / 8KB stores).

Per group (prep) and per pair of 128-row tiles (software-pipelined: the
S stage for pair u+1 is emitted before the M stage of pair u, so the PE
never stalls on the transpose->copy->load-weights chain):
  - prep: one DMA load each for indices/weights [128,8,8]; DVE row totals
    -> reciprocals; normalized bf16 weights w' = w/total in one
    scalar_tensor_tensor with a zero-stride broadcast of the reciprocals;
    scatter offsets offs = 512*(t%2) + 64*k + idx (int16) in one int add
  - S: GPSIMD local_scatter builds one-hot rows
    eqw[b, (t%2)*512 + 64*k + e] = w'[b,k] (collision-free by k-slot);
    DVE 3-op bf16 add-tree reduces k -> agg2 [128, 2, 64]; PE transpose
    (identity matmul) -> psum; DVE copy -> aggT [128,128] bf16 SBUF
  - M: per tile h: PE matmuls path = aggT_h.T @ V (N=512, psum f32) and
    m1 = aggT_h.T @ G (G = V V^T, N=64); DVE scalar_tensor_tensor
    accumulates sqnorm[b] = sum_e m1[b,e]*agg[b,e] = ||path_b||^2; ACT
    copies psum -> SBUF bf16 (already normalized); one 1MB store per group
Epilogue: norm = sqrt(sqnorm); per-partition partial sums -> "eff".
Host: concat path shards (bf16 -> f32), sum efficiency partials / B.

Measured on 8 axon-tunneled TRN2 NeuronCores: ~77-95 us whole-NEFF
exec_time_ns (run-to-run device variance), path rel-err 2.9e-3,
efficiency rel-err 1e-4 (bf16 table/weights; f32 accumulation).
"""

import numpy as np

N_CORES = 8
B_FULL = 65536
B_CORE = B_FULL // N_CORES  # 8192
K = 8
E = 64
D = 512
GT = 8  # tiles per group
NG = B_CORE // (128 * GT)  # 8 groups
NT = B_CORE // 128  # 64 tiles

_CACHE = {}


def _build():
    import concourse.bacc as bacc
    import concourse.mybir as mybir
    import concourse.tile as tile
    from concourse import masks

    f32 = mybir.dt.float32
    bf16 = mybir.dt.bfloat16
    i32 = mybir.dt.int32
    i16 = mybir.dt.int16
    ALU = mybir.AluOpType
    AF = mybir.ActivationFunctionType

    nc = bacc.Bacc(None, target_bir_lowering=False, debug=False)

    idx_d = nc.declare_dram_parameter("expert_indices", [B_CORE, K], i32, isOutput=False)
    w_d = nc.declare_dram_parameter("expert_weights", [B_CORE, K], f32, isOutput=False)
    # vertices pre-cast to bf16 on host; stacked twice so both PE partition
    # halves hold a copy (lhsT base partition must match rhs base partition).
    v_d = nc.declare_dram_parameter("vertices_bf", [2, E, D], bf16, isOutput=False)
    vt_d = nc.declare_dram_parameter("vertices_t_bf", [D, E], bf16, isOutput=False)
    path_d = nc.declare_dram_parameter("path", [B_CORE, D], bf16, isOutput=True)
    eff_d = nc.declare_dram_parameter("eff", [128, 1], f32, isOutput=True)

    with tile.TileContext(nc) as tc:
        with (
            tc.tile_pool(name="const", bufs=1) as cpool,
            tc.tile_pool(name="work", bufs=6) as pool,
            tc.tile_pool(name="prep", bufs=NG) as prpool,
            tc.tile_pool(name="aggTp", bufs=8) as apool,
            tc.tile_pool(name="out", bufs=5) as opool,
            tc.tile_pool(name="ppair", bufs=2, space="PSUM") as ppool,
            tc.tile_pool(name="ptrans", bufs=2, space="PSUM") as tpool,
            tc.tile_pool(name="pm1", bufs=2, space="PSUM") as mpool,
        ):
            # ---- constants ----
            vb = cpool.tile([128, D], bf16)
            nc.sync.dma_start(vb[:], v_d.rearrange("two e d -> (two e) d"))
            vtb = cpool.tile([128, 4, E], bf16)
            nc.sync.dma_start(vtb[:], vt_d.rearrange("(c p) e -> p c e", p=128))

            # scatter offset bias first: the first local_scatter depends on it
            # (via offs), so keep it at the head of the gpsimd stream
            kvec = cpool.tile([128, GT, K], i32)
            nc.gpsimd.iota(
                kvec[:], pattern=[[0, GT // 2], [512, 2], [64, K]], base=0,
                channel_multiplier=0,
            )

            ident = cpool.tile([128, 128], bf16)
            masks.make_identity(nc, ident[:])

            # G = V @ V.T  [64, 64], replicated into both partition halves
            g_ps = ppool.tile([E, E], f32, tag="pps")
            for c in range(4):
                nc.tensor.matmul(
                    g_ps[:], vtb[:, c, :], vtb[:, c, :], start=(c == 0), stop=(c == 3)
                )
            gb = cpool.tile([128, E], bf16)
            nc.vector.tensor_copy(gb[0:64, :], g_ps[:])
            nc.vector.tensor_copy(gb[64:128, :], g_ps[:])

            recips = cpool.tile([128, NT], f32)
            sqnorms = cpool.tile([128, NT], f32)

            NPAIR = NT // 2
            PPG = GT // 2  # pairs per group
            state = {}  # per-group tiles

            def group_prep(g):
                r0 = g * 128 * GT
                rows = slice(r0, r0 + 128 * GT)
                idx8 = prpool.tile([128, GT, K], i32)
                nc.sync.dma_start(idx8[:], idx_d[rows, :].rearrange("(p t) k -> p t k", p=128))
                w8 = prpool.tile([128, GT, K], f32)
                nc.sync.dma_start(w8[:], w_d[rows, :].rearrange("(p t) k -> p t k", p=128))

                tot8 = prpool.tile([128, GT], f32)
                nc.vector.tensor_reduce(tot8[:], w8[:], axis=mybir.AxisListType.X, op=ALU.add)
                nc.vector.reciprocal(recips[:, g * GT : (g + 1) * GT], tot8[:])

                # normalize during the bf16 cast: w' = w / total
                # (reciprocals broadcast over k via a zero-stride AP)
                w8b = prpool.tile([128, GT, K], bf16)
                rb = recips[:, g * GT : (g + 1) * GT].unsqueeze(2).broadcast_to([128, GT, K])
                nc.vector.scalar_tensor_tensor(
                    out=w8b[:], in0=w8[:], scalar=1.0, in1=rb,
                    op0=ALU.mult, op1=ALU.mult,
                )
                offs = prpool.tile([128, GT, K], i16)
                nc.vector.tensor_tensor(offs[:], idx8[:], kvec[:], op=ALU.add)
                state[g] = {"w8b": w8b, "offs": offs}

            qstate = {}  # quad-index -> eqw tile

            def stage_s(up):
                """scatter for pair up; on quad completion: tree + 2x transpose"""
                g, u = divmod(up, PPG)
                st = state[g]
                pair = slice(2 * u, 2 * u + 2)
                q, v = divmod(up, 2)
                if v == 0:
                    eqw = pool.tile([128, 4, 512], bf16)
                    qstate[q] = eqw
                else:
                    eqw = qstate[q]
                nc.gpsimd.local_scatter(
                    eqw[:, 2 * v : 2 * v + 2, :].rearrange("p t c -> p (t c)"),
                    st["w8b"][:, pair, :].rearrange("p t k -> p (t k)"),
                    st["offs"][:, pair, :].rearrange("p t k -> p (t k)"),
                    channels=128,
                    num_elems=1024,
                    num_idxs=16,
                )
                if v == 0:
                    return
                # quad complete: k-reduction tree across all 4 tiles at once
                eqw = qstate.pop(q)
                s1 = pool.tile([128, 4, 256], bf16)
                nc.vector.tensor_tensor(s1[:], eqw[:, :, 0:256], eqw[:, :, 256:512], op=ALU.add)
                s2 = pool.tile([128, 4, 128], bf16)
                nc.vector.tensor_tensor(s2[:], s1[:, :, 0:128], s1[:, :, 128:256], op=ALU.add)
                agg4 = pool.tile([128, 4, E], bf16)
                nc.vector.tensor_tensor(agg4[:], s2[:, :, 0:64], s2[:, :, 64:128], op=ALU.add)

                for w in range(2):
                    pu = 2 * q + w
                    gg, uu = divmod(pu, PPG)
                    agg2 = agg4[:, 2 * w : 2 * w + 2, :]
                    psT = tpool.tile([128, 128], bf16)
                    nc.tensor.transpose(psT[:], agg2.rearrange("p t e -> p (t e)"), ident[:])
                    aggT = apool.tile([128, 128], bf16)
                    nc.vector.tensor_copy(aggT[:], psT[:])
                    state[gg][uu] = (aggT, agg2)

            def stage_m(up):
                """matmuls + sqnorm + psum->sbuf copy (+ store at group end)"""
                g, u = divmod(up, PPG)
                st = state[g]
                aggT, agg2 = st.pop(u)
                pair = slice(2 * u, 2 * u + 2)
                if u == 0:
                    osb = opool.tile([128, GT, D], bf16, name="osb", tag="osb")
                    st["osb"] = osb
                osb = st["osb"]
                pps = ppool.tile([128, 2, D], f32, tag="pps")
                m1 = mpool.tile([128, 2, E], f32)
                # NOTE: keep path/m1 matmuls interleaved per half — clustering
                # the two path matmuls (disjoint row-groups) crashes the
                # device deterministically (NRT INTERNAL error).
                for h in range(2):
                    col = g * GT + 2 * u + h
                    half = slice(64 * h, 64 * h + 64)
                    nc.tensor.matmul(pps[:, h, :], aggT[half, :], vb[half, :], start=True, stop=True)
                    nc.tensor.matmul(m1[:, h, :], aggT[half, :], gb[half, :], start=True, stop=True)
                    scr = pool.tile([128, E], bf16)
                    nc.vector.scalar_tensor_tensor(
                        out=scr[:],
                        in0=m1[:, h, :],
                        scalar=1.0,
                        in1=agg2[:, h, :],
                        op0=ALU.mult,
                        op1=ALU.mult,
                        accum_out=sqnorms[:, col : col + 1],
                    )
                # agg already normalized -> plain pair-wide psum->sbuf copy
                nc.scalar.activation(osb[:, pair, :], pps[:], AF.Copy, bias=0.0)
                if u % 2 == 1:
                    # store half a group (4 tiles = 4KB contiguous/partition)
                    v = u // 2
                    r0 = g * 128 * GT
                    rows = slice(r0, r0 + 128 * GT)
                    nc.sync.dma_start(
                        path_d[rows, :].rearrange("(p t) d -> p t d", p=128)[
                            :, 4 * v : 4 * v + 4, :
                        ],
                        osb[:, 4 * v : 4 * v + 4, :],
                    )
                    if u == PPG - 1:
                        del state[g]["osb"]

            # software pipeline: S runs two pairs ahead of M
            LOOKAHEAD = 2
            group_prep(0)
            for p0 in range(LOOKAHEAD):
                if p0 % PPG == 0 and p0 > 0:
                    group_prep(p0 // PPG)
                stage_s(p0)
            for up in range(NPAIR):
                nu = up + LOOKAHEAD
                if nu < NPAIR:
                    if nu % PPG == 0:
                        group_prep(nu // PPG)
                    stage_s(nu)
                stage_m(up)

            # ---- efficiency epilogue ----
            # agg was pre-normalized, so sqnorms[b] is already ||path_b||^2
            norms = cpool.tile([128, NT], f32)
            nc.scalar.activation(norms[:], sqnorms[:], AF.Sqrt)
            effp = cpool.tile([128, 1], f32)
            nc.vector.tensor_reduce(effp[:], norms[:], axis=mybir.AxisListType.X, op=ALU.add)
            nc.sync.dma_start(eff_d[:], effp[:])

    nc.compile()
    return nc


def _get_nc():
    if "nc" not in _CACHE:
        _CACHE["nc"] = _build()
    return _CACHE["nc"]


def _run(in_maps, trace=False):
    from concourse.bass_utils import run_bass_kernel_spmd

    nc = _get_nc()
    return run_bass_kernel_spmd(nc, in_maps, list(range(N_CORES)), trace=trace)


def _make_in_maps(expert_indices, expert_weights, vertices):
    import ml_dtypes

    idx = np.ascontiguousarray(np.asarray(expert_indices, dtype=np.int32))
    w = np.ascontiguousarray(np.asarray(expert_weights, dtype=np.float32))
    v = np.asarray(vertices, dtype=np.float32)
    vbf = np.ascontiguousarray(
        np.broadcast_to(v.astype(ml_dtypes.bfloat16), (2, E, D))
    )
    vtbf = np.ascontiguousarray(v.T.astype(ml_dtypes.bfloat16))
    in_maps = []
    for i in range(N_CORES):
        s = slice(i * B_CORE, (i + 1) * B_CORE)
        in_maps.append(
            {
                "expert_indices": np.ascontiguousarray(idx[s]),
                "expert_weights": np.ascontiguousarray(w[s]),
                "vertices_bf": vbf,
                "vertices_t_bf": vtbf,
            }
        )
    return in_maps


def _assemble(results):
    path = np.concatenate(
        [np.asarray(r["path"]).astype(np.float32) for r in results], axis=0
    )
    eff = sum(float(np.asarray(r["eff"], dtype=np.float64).sum()) for r in results)
    eff = np.float32(eff / B_FULL)
    return path, eff


def kernel(expert_indices, expert_weights, vertices):
    in_maps = _make_in_maps(expert_indices, expert_weights, vertices)
    last_err = None
    for attempt in range(3):
        try:
            res = _run(in_maps, trace=False)
            return _assemble(res.results)
        except Exception as e:  # rare transient device errors -> retry
            last_err = e
            _CACHE.clear()
    raise last_err


# revision 66
# speedup vs baseline: 1.2337x; 1.1802x over previous
"""Trainium2 Bass kernel for nn_Amplituedro (weighted embedding lookup).

path[b] = (sum_k w[b,k] * vertices[idx[b,k]]) / sum_k w[b,k]
eff     = mean_b ||path[b]||

Data-parallel over batch: 8 cores x 8192 rows. Per core, 8 groups of
8 x 128-row tiles with an interleaved row mapping (row = g*1024 + p*8 + t)
so per-partition DRAM accesses are contiguous (256B loads / 8KB stores).

Per group (prep) and per pair of 128-row tiles (software-pipelined: the
S stage for pair u+1 is emitted before the M stage of pair u, so the PE
never stalls on the transpose->copy->load-weights chain):
  - prep: one DMA load each for indices/weights [128,8,8]; DVE row totals
    -> reciprocals; normalized bf16 weights w' = w/total in one
    scalar_tensor_tensor with a zero-stride broadcast of the reciprocals;
    scatter offsets offs = 512*(t%2) + 64*k + idx (int16) in one int add
  - S: GPSIMD local_scatter builds one-hot rows
    eqw[b, (t%2)*512 + 64*k + e] = w'[b,k] (collision-free by k-slot);
    DVE 3-op bf16 add-tree reduces k -> agg2 [128, 2, 64]; PE transpose
    (identity matmul) -> psum; DVE copy -> aggT [128,128] bf16 SBUF
  - M: per tile h: PE matmuls path = aggT_h.T @ V (N=512, psum f32) and
    m1 = aggT_h.T @ G (G = V V^T, N=64); DVE scalar_tensor_tensor
    accumulates sqnorm[b] = sum_e m1[b,e]*agg[b,e] = ||path_b||^2; ACT
    copies psum -> SBUF bf16 (already normalized); one 1MB store per group
Epilogue: norm = sqrt(sqnorm); per-partition partial sums -> "eff".
Host: concat path shards (bf16 -> f32), sum efficiency partials / B.

Measured on 8 axon-tunneled TRN2 NeuronCores: ~77-95 us whole-NEFF
exec_time_ns (run-to-run device variance), path rel-err 2.9e-3,
efficiency rel-err 1e-4 (bf16 table/weights; f32 accumulation).
"""

import numpy as np

N_CORES = 8
B_FULL = 65536
B_CORE = B_FULL // N_CORES  # 8192
K = 8
E = 64
D = 512
GT = 8  # tiles per group
NG = B_CORE // (128 * GT)  # 8 groups
NT = B_CORE // 128  # 64 tiles

_CACHE = {}


def _build():
    import concourse.bacc as bacc
    import concourse.mybir as mybir
    import concourse.tile as tile
    from concourse import masks

    f32 = mybir.dt.float32
    bf16 = mybir.dt.bfloat16
    i32 = mybir.dt.int32
    i16 = mybir.dt.int16
    ALU = mybir.AluOpType
    AF = mybir.ActivationFunctionType

    nc = bacc.Bacc(None, target_bir_lowering=False, debug=False)

    idx_d = nc.declare_dram_parameter("expert_indices", [B_CORE, K], i32, isOutput=False)
    w_d = nc.declare_dram_parameter("expert_weights", [B_CORE, K], f32, isOutput=False)
    # vertices pre-cast to bf16 on host; stacked twice so both PE partition
    # halves hold a copy (lhsT base partition must match rhs base partition).
    v_d = nc.declare_dram_parameter("vertices_bf", [2, E, D], bf16, isOutput=False)
    vt_d = nc.declare_dram_parameter("vertices_t_bf", [D, E], bf16, isOutput=False)
    path_d = nc.declare_dram_parameter("path", [B_CORE, D], bf16, isOutput=True)
    eff_d = nc.declare_dram_parameter("eff", [128, 1], f32, isOutput=True)

    with tile.TileContext(nc) as tc:
        with (
            tc.tile_pool(name="const", bufs=1) as cpool,
            tc.tile_pool(name="work", bufs=6) as pool,
            tc.tile_pool(name="prep", bufs=NG) as prpool,
            tc.tile_pool(name="aggTp", bufs=8) as apool,
            tc.tile_pool(name="out", bufs=5) as opool,
            tc.tile_pool(name="ppair", bufs=2, space="PSUM") as ppool,
            tc.tile_pool(name="ptrans", bufs=2, space="PSUM") as tpool,
            tc.tile_pool(name="pm1", bufs=2, space="PSUM") as mpool,
        ):
            # ---- constants ----
            vb = cpool.tile([128, D], bf16)
            nc.sync.dma_start(vb[:], v_d.rearrange("two e d -> (two e) d"))
            vtb = cpool.tile([128, 4, E], bf16)
            nc.sync.dma_start(vtb[:], vt_d.rearrange("(c p) e -> p c e", p=128))

            # scatter offset bias first: the first local_scatter depends on it
            # (via offs), so keep it at the head of the gpsimd stream
            kvec = cpool.tile([128, GT, K], i32)
            nc.gpsimd.iota(
                kvec[:], pattern=[[0, GT // 2], [512, 2], [64, K]], base=0,
                channel_multiplier=0,
            )

            ident = cpool.tile([128, 128], bf16)
            masks.make_identity(nc, ident[:])

            # G = V @ V.T  [64, 64], replicated into both partition halves
            g_ps = ppool.tile([E, E], f32, tag="pps")
            for c in range(4):
                nc.tensor.matmul(
                    g_ps[:], vtb[:, c, :], vtb[:, c, :], start=(c == 0), stop=(c == 3)
                )
            gb = cpool.tile([128, E], bf16)
            nc.vector.tensor_copy(gb[0:64, :], g_ps[:])
            nc.vector.tensor_copy(gb[64:128, :], g_ps[:])

            recips = cpool.tile([128, NT], f32)
            sqnorms = cpool.tile([128, NT], f32)

            NPAIR = NT // 2
            PPG = GT // 2  # pairs per group
            state = {}  # per-group tiles

            def group_prep(g):
                r0 = g * 128 * GT
                rows = slice(r0, r0 + 128 * GT)
                idx8 = prpool.tile([128, GT, K], i32)
                nc.sync.dma_start(idx8[:], idx_d[rows, :].rearrange("(p t) k -> p t k", p=128))
                w8 = prpool.tile([128, GT, K], f32)
                nc.sync.dma_start(w8[:], w_d[rows, :].rearrange("(p t) k -> p t k", p=128))

                tot8 = prpool.tile([128, GT], f32)
                nc.vector.tensor_reduce(tot8[:], w8[:], axis=mybir.AxisListType.X, op=ALU.add)
                nc.vector.reciprocal(recips[:, g * GT : (g + 1) * GT], tot8[:])

                # normalize during the bf16 cast: w' = w / total
                # (reciprocals broadcast over k via a zero-stride AP)
                w8b = prpool.tile([128, GT, K], bf16)
                rb = recips[:, g * GT : (g + 1) * GT].unsqueeze(2).broadcast_to([128, GT, K])
                nc.vector.scalar_tensor_tensor(
                    out=w8b[:], in0=w8[:], scalar=1.0, in1=rb,
                    op0=ALU.mult, op1=ALU.mult,
                )
                offs = prpool.tile([128, GT, K], i16)
                nc.vector.tensor_tensor(offs[:], idx8[:], kvec[:], op=ALU.add)
                state[g] = {"w8b": w8b, "offs": offs}

            qstate = {}  # quad-index -> eqw tile

            def stage_s(up):
                """scatter for pair up; on quad completion: tree + 2x transpose"""
                g, u = divmod(up, PPG)
                st = state[g]
                pair = slice(2 * u, 2 * u + 2)
                q, v = divmod(up, 2)
                if v == 0:
                    eqw = pool.tile([128, 4, 512], bf16)
                    qstate[q] = eqw
                else:
                    eqw = qstate[q]
                nc.gpsimd.local_scatter(
                    eqw[:, 2 * v : 2 * v + 2, :].rearrange("p t c -> p (t c)"),
                    st["w8b"][:, pair, :].rearrange("p t k -> p (t k)"),
                    st["offs"][:, pair, :].rearrange("p t k -> p (t k)"),
                    channels=128,
                    num_elems=1024,
                    num_idxs=16,
                )
                if v == 0:
                    return
                # quad complete: k-reduction tree across all 4 tiles at once
                eqw = qstate.pop(q)
                s1 = pool.tile([128, 4, 256], bf16)
                nc.vector.tensor_tensor(s1[:], eqw[:, :, 0:256], eqw[:, :, 256:512], op=ALU.add)
                s2 = pool.tile([128, 4, 128], bf16)
                nc.vector.tensor_tensor(s2[:], s1[:, :, 0:128], s1[:, :, 128:256], op=ALU.add)
                agg4 = pool.tile([128, 4, E], bf16)
                nc.vector.tensor_tensor(agg4[:], s2[:, :, 0:64], s2[:, :, 64:128], op=ALU.add)

                for w in range(2):
                    pu = 2 * q + w
                    gg, uu = divmod(pu, PPG)
                    agg2 = agg4[:, 2 * w : 2 * w + 2, :]
                    psT = tpool.tile([128, 128], bf16)
                    nc.tensor.transpose(psT[:], agg2.rearrange("p t e -> p (t e)"), ident[:])
                    aggT = apool.tile([128, 128], bf16)
                    nc.vector.tensor_copy(aggT[:], psT[:])
                    state[gg][uu] = (aggT, agg2)

            def stage_m(up):
                """matmuls + sqnorm + psum->sbuf copy (+ store at group end)"""
                g, u = divmod(up, PPG)
                st = state[g]
                aggT, agg2 = st.pop(u)
                pair = slice(2 * u, 2 * u + 2)
                if u == 0:
                    osb = opool.tile([128, GT, D], bf16, name="osb", tag="osb")
                    st["osb"] = osb
                osb = st["osb"]
                pps = ppool.tile([128, 2, D], f32, tag="pps")
                m1 = mpool.tile([128, 2, E], f32)
                # NOTE: keep path/m1 matmuls interleaved per half — clustering
                # the two path matmuls (disjoint row-groups) crashes the
                # device deterministically (NRT INTERNAL error).
                for h in range(2):
                    col = g * GT + 2 * u + h
                    half = slice(64 * h, 64 * h + 64)
                    nc.tensor.matmul(pps[:, h, :], aggT[half, :], vb[half, :], start=True, stop=True)
                    nc.tensor.matmul(m1[:, h, :], aggT[half, :], gb[half, :], start=True, stop=True)
                    scr = pool.tile([128, E], bf16)
                    nc.vector.scalar_tensor_tensor(
                        out=scr[:],
                        in0=m1[:, h, :],
                        scalar=1.0,
                        in1=agg2[:, h, :],
                        op0=ALU.mult,
                        op1=ALU.mult,
                        accum_out=sqnorms[:, col : col + 1],
                    )
                # agg already normalized -> plain pair-wide psum->sbuf copy
                nc.scalar.activation(osb[:, pair, :], pps[:], AF.Copy, bias=0.0)
                if u % 2 == 1:
                    # store half a group (4 tiles = 4KB contiguous/partition)
                    v = u // 2
                    r0 = g * 128 * GT
                    rows = slice(r0, r0 + 128 * GT)
                    nc.sync.dma_start(
                        path_d[rows, :].rearrange("(p t) d -> p t d", p=128)[
                            :, 4 * v : 4 * v + 4, :
                        ],
                        osb[:, 4 * v : 4 * v + 4, :],
                    )
                    if u == PPG - 1:
                        del state[g]["osb"]

            # software pipeline: S runs two pairs ahead of M
            LOOKAHEAD = 2
            group_prep(0)
            for p0 in range(LOOKAHEAD):
                if p0 % PPG == 0 and p0 > 0:
                    group_prep(p0 // PPG)
                stage_s(p0)
            for up in range(NPAIR):
                nu = up + LOOKAHEAD
                if nu < NPAIR:
                    if nu % PPG == 0:
                        group_prep(nu // PPG)
                    stage_s(nu)
                stage_m(up)

            # ---- efficiency epilogue ----
            # agg was pre-normalized, so sqnorms[b] is already ||path_b||^2
            norms = cpool.tile([128, NT], f32)
            nc.scalar.activation(norms[:], sqnorms[:], AF.Sqrt)
            effp = cpool.tile([128, 1], f32)
            nc.vector.tensor_reduce(effp[:], norms[:], axis=mybir.AxisListType.X, op=ALU.add)
            nc.sync.dma_start(eff_d[:], effp[:])

    nc.compile()
    return nc


def _get_nc():
    if "nc" not in _CACHE:
        _CACHE["nc"] = _build()
    return _CACHE["nc"]


def _run(in_maps, trace=False):
    from concourse.bass_utils import run_bass_kernel_spmd

    nc = _get_nc()
    return run_bass_kernel_spmd(nc, in_maps, list(range(N_CORES)), trace=trace)


def _make_in_maps(expert_indices, expert_weights, vertices):
    import ml_dtypes

    idx = np.ascontiguousarray(np.asarray(expert_indices, dtype=np.int32))
    w = np.ascontiguousarray(np.asarray(expert_weights, dtype=np.float32))
    v = np.asarray(vertices, dtype=np.float32)
    vbf = np.ascontiguousarray(
        np.broadcast_to(v.astype(ml_dtypes.bfloat16), (2, E, D))
    )
    vtbf = np.ascontiguousarray(v.T.astype(ml_dtypes.bfloat16))
    in_maps = []
    for i in range(N_CORES):
        s = slice(i * B_CORE, (i + 1) * B_CORE)
        in_maps.append(
            {
                "expert_indices": np.ascontiguousarray(idx[s]),
                "expert_weights": np.ascontiguousarray(w[s]),
                "vertices_bf": vbf,
                "vertices_t_bf": vtbf,
            }
        )
    return in_maps


def _assemble(results):
    path = np.concatenate(
        [np.asarray(r["path"]).astype(np.float32) for r in results], axis=0
    )
    eff = sum(float(np.asarray(r["eff"], dtype=np.float64).sum()) for r in results)
    eff = np.float32(eff / B_FULL)
    return path, eff


def kernel(expert_indices, expert_weights, vertices):
    in_maps = _make_in_maps(expert_indices, expert_weights, vertices)
    last_err = None
    for attempt in range(3):
        try:
            res = _run(in_maps, trace=False)
            return _assemble(res.results)
        except Exception as e:  # rare transient device errors -> retry
            last_err = e
            _CACHE.clear()
    raise last_err
